# revision 1
# baseline (speedup 1.0000x reference)
"""Bahdanau additive-attention kernel for Trainium2, SPMD across 8 NeuronCores.

Reference computation (all fp32):
    q_proj  = query @ W1_w.T + W1_b            # [D]
    v_proj  = values @ W2_w.T + W2_b           # [T, D]
    weights = softmax(tanh(q_proj + v_proj) * v, axis=0)   # over T
    out     = weights * values                 # [T, D]

Sharding: values is split along T across 8 cores (2048 rows each); W2 is
replicated (shipped pre-transposed + pre-blocked in bf16); the q-projection
matvec is sharded over the contraction dim (each core handles 256 columns of
W1) and finished with an AllReduce; the softmax denominator (per-column sum
of exps) is AllReduced.  Logits are bounded in [-0.1, 0.1] (tanh * v with
|v| <= 0.1) so the softmax needs no max-subtraction pass.

Per-core device program:
  - VT (values shard transposed, bf16) resident in SBUF as the moving matmul
    operand; psum tiles are [d=128 part, t=512 free]; the k loop is OUTER so
    the first tiles stream at DMA pace and the stationary operand is reused
    across 4 consecutive matmuls.
  - ScalarE: tanh(psum + qb[d]) then exp(v[d] * x) with accum_out giving the
    per-partition running sum of exps (softmax denominator) for free.
  - e stored fp16 in SBUF.  Pass 2: e *= 1/S[d] (per-partition tensor_scalar,
    in place), outT = e * valuesT(fp32) on DVE, TensorE transposes outT back
    to [t, d], ScalarE evacuates PSUM to SBUF, DMA out.
"""

import numpy as np

import concourse.bacc as bacc
import concourse.bass as bass
import concourse.tile as tile
from concourse import mybir
from concourse import masks
from concourse.bass_utils import run_bass_kernel_spmd

F32 = mybir.dt.float32
BF16 = mybir.dt.bfloat16
FP16 = mybir.dt.float16
FP8 = mybir.dt.float8e4

D = 2048          # feature dim
T = 16384         # total timesteps
N_CORES = 8
TS = T // N_CORES  # timesteps per core = 2048
KS = D // N_CORES  # W1 contraction slice per core = 256


def build_kernel(D=D, TS=TS, KS=KS, n_cores=N_CORES, debug=False):
    DT = D // 128     # d-tiles of 128
    KT = D // 128     # k-tiles of 128
    TC = TS // 512    # t-chunks of 512
    IT = TS // 128    # t-tiles of 128
    GJ = min(4, DT)   # dj per pass-2 group (one 512-wide d-chunk)
    NG = DT // GJ     # number of pass-2 groups
    THW = min(1024, TS)  # pass-2 t-half width
    NTH = TS // THW
    N_CORES_ = n_cores

    nc = bacc.Bacc(None, target_bir_lowering=False, debug=debug, num_devices=N_CORES_)

    # Per-core inputs (see make_in_maps for host-side layouts)
    valsT = nc.dram_tensor("valsT", [D, TS], FP16, kind="ExternalInput")
    w2t = nc.dram_tensor("w2t", [DT, 128, KT * 128], FP16, kind="ExternalInput")
    w1t_d = nc.dram_tensor("w1t_d", [KT, 128, D], FP8, kind="ExternalInput")
    qfull = nc.dram_tensor("qfull", [D], F32, kind="ExternalInput")
    w1b = nc.dram_tensor("w1b", [D], F32, kind="ExternalInput")
    w2b = nc.dram_tensor("w2b", [D], F32, kind="ExternalInput")
    vvec = nc.dram_tensor("vvec", [D], F32, kind="ExternalInput")
    out = nc.dram_tensor("out", [TS, D], F32, kind="ExternalOutput")

    with tile.TileContext(nc) as tc:
        with (
            tc.tile_pool(name="const", bufs=1) as const_pool,
            tc.tile_pool(name="vt", bufs=1) as vt_pool,
            tc.tile_pool(name="e", bufs=1) as e_pool,
            tc.tile_pool(name="w2tb", bufs=2) as w2tb_pool,
            tc.tile_pool(name="st", bufs=2) as st_pool,
            tc.tile_pool(name="outT", bufs=6) as outT_pool,
            tc.tile_pool(name="osb", bufs=8) as osb_pool,
            tc.tile_pool(name="stg", bufs=8) as stg_pool,
            tc.tile_pool(name="psum", bufs=6, space="PSUM") as psum_pool,
            tc.tile_pool(name="psumT", bufs=2, space="PSUM") as psumT_pool,
            tc.tile_pool(name="dram", bufs=1, space="DRAM") as dram_pool,
        ):
            # ---------------- constants / small vectors ----------------
            qbv = const_pool.tile([128, DT], F32)    # qb[d] laid out [p, dj]
            vv = const_pool.tile([128, DT], F32)     # v[d]
            rv2 = const_pool.tile([128, DT], F32)    # 2^14 / S[d]
            Sloc = const_pool.tile([128, DT], F32)   # local sum-exp
            b1v = const_pool.tile([128, DT], F32)
            b2v = const_pool.tile([128, DT], F32)
            acc = const_pool.tile([128, DT * TC], F32)  # per (dj, tc) exp-sums
            ident16 = const_pool.tile([128, 128], FP16)
            ones1 = const_pool.tile([1, 128], F32)
            qs1 = const_pool.tile([1, KS], F32)
            qpart = const_pool.tile([128, DT], F32)  # local q_proj partial

            masks.make_identity(nc, ident16[:, :])
            nc.vector.memset(ones1[:, :], 1.0)

            DH = (3 * DT // 4) if DT >= 4 else DT

            # ---------------- warmup collective (absorbs ncfw first-use) ----
            wu_in = dram_pool.tile([1, 32], F32, name="wu_in")
            wu_out = dram_pool.tile([1, 32], F32, name="wu_out")
            wuz = const_pool.tile([1, 32], F32)
            nc.vector.memset(wuz[:, :], 0.0)
            nc.gpsimd.dma_start(wu_in[:, :], wuz[:, :])
            nc.gpsimd.collective_compute(
                "AllReduce", mybir.AluOpType.add,
                replica_groups=[list(range(N_CORES_))],
                ins=[wu_in.opt()], outs=[wu_out.opt()],
            )

            # first two W2T blocks land before the VT bulk so dj0 matmuls
            # can start immediately
            w2tb_pre = []
            for i in range(min(2, DT)):
                wpre = w2tb_pool.tile([128, KT * 128], FP16, tag="w2tb",
                                      name=f"w2tbp{i}")
                nc.sync.dma_start(wpre[:, :], w2t[i, :, :])
                w2tb_pre.append(wpre)

            # ---------------- VT resident load (fp16) -------------------
            # vt[kt][p, t] = values_s[t, 128*kt + p]
            vt_tiles = []
            VH = TS // 2
            for kt in range(KT):
                vt = vt_pool.tile([128, TS], FP16, name=f"vt{kt}")
                vt_tiles.append(vt)
            for half in range(2):
                for kt in range(KT):
                    eng = nc.sync if kt % 2 == 0 else nc.scalar
                    eng.dma_start(
                        vt_tiles[kt][:, half * VH:(half + 1) * VH],
                        valsT[kt * 128:(kt + 1) * 128, half * VH:(half + 1) * VH])

            # ---------------- pass 1: matmul + tanh + exp ---------------
            e_tiles = []
            for dj in range(DT):
                e_tiles.append(e_pool.tile([128, TS], FP16, name=f"e{dj}"))

            qcol = const_pool.tile([128, KT], F32)   # q in [p, kt] layout
            qcol16 = const_pool.tile([128, KT], FP8)
            qrow = const_pool.tile([1, D], F32)      # q_proj as a row
            nc.gpsimd.dma_start(qcol[:, :], qfull[:].rearrange("(kt p) -> p kt", p=128))
            nc.vector.tensor_copy(qcol16[:, :], qcol[:, :])
            QW = min(512, D)
            QDC = D // QW

            def emit_matvec(w1_pool):
                # q_proj row = sum_kt q_col[kt].T @ W1T[kt] in fp8 (softmax is
                # invariant to the per-column q_proj quantization error), then
                # transpose the row into the per-partition [p, dj] layout.
                if True:
                    pq_tiles = [psum_pool.tile([1, QW], F32, name=f"pq{dc}", tag="ps")
                                for dc in range(QDC)]
                    for kt in range(KT):
                        w1tile = w1_pool.tile([128, D], FP8, tag="w1t")
                        nc.gpsimd.dma_start(w1tile[:, :], w1t_d[kt, :, :])
                        for dc in range(QDC):
                            nc.tensor.matmul(
                                pq_tiles[dc][:, :], qcol16[:, kt:kt + 1],
                                w1tile[:, dc * QW:(dc + 1) * QW],
                                start=(kt == 0), stop=(kt == KT - 1))
                    for dc in range(QDC):
                        nc.scalar.copy(qrow[:, dc * QW:(dc + 1) * QW], pq_tiles[dc][:, :])
                pqt = psumT_pool.tile([128, DT], F32, name="pqt", tag="pT")
                for dj in range(DT):
                    nc.tensor.transpose(
                        pqt[:, dj:dj + 1],
                        qrow[:, dj * 128:(dj + 1) * 128], ones1[:, 0:1])
                nc.scalar.copy(qbv[:, :], pqt[:, :])

                # biases / v in [p, dj] layout: elem (p, j) <- dram[128j + p]
                nc.gpsimd.dma_start(b1v[:, :], w1b[:].rearrange("(j p) -> p j", p=128))
                nc.gpsimd.dma_start(b2v[:, :], w2b[:].rearrange("(j p) -> p j", p=128))
                nc.gpsimd.dma_start(vv[:, :], vvec[:].rearrange("(j p) -> p j", p=128))
                nc.vector.tensor_add(b1v[:, :], b1v[:, :], b2v[:, :])
                nc.vector.tensor_add(qbv[:, :], qbv[:, :], b1v[:, :])


            ndma_state = [0]

            def emit_group(djs, dual_issue=False):
                # pass-2 pipeline for a list of dj tiles (one contiguous
                # output chunk): scale e by 2^14/S, multiply with resident
                # fp16 VT, transpose on TensorE, descale-evacuate on ScalarE.
                nj = len(djs)
                d0 = djs[0]
                for th in range(NTH):
                    oT = []
                    for jj in range(nj):
                        dj = djs[jj]
                        if th == 0:
                            nc.vector.tensor_scalar(
                                out=e_tiles[dj][:, :], in0=e_tiles[dj][:, :],
                                scalar1=rv2[:, dj:dj + 1], scalar2=None,
                                op0=mybir.AluOpType.mult)
                        ot = outT_pool.tile([128, THW], FP16, tag="oT", name="ot")
                        nc.vector.tensor_mul(
                            ot[:, :],
                            e_tiles[dj][:, th * THW:(th + 1) * THW],
                            vt_tiles[dj][:, th * THW:(th + 1) * THW])
                        oT.append(ot)
                    for itl in range(THW // 128):
                        it = th * (THW // 128) + itl
                        pst = psumT_pool.tile([128, nj * 128], FP16, tag="pT",
                                              name="pst")
                        for jj in range(nj):
                            nc.tensor.transpose(
                                pst[:, jj * 128:(jj + 1) * 128],
                                oT[jj][:, itl * 128:(itl + 1) * 128],
                                ident16[:, :],
                            )
                        osb = osb_pool.tile([128, nj * 128], F32, name="osb",
                                            tag="osb")
                        nc.scalar.activation(
                            osb[:, :], pst[:, :],
                            mybir.ActivationFunctionType.Copy,
                            bias=0.0, scale=0.00006103515625)
                        ndma_state[0] += 1
                        eng = nc.gpsimd if (dual_issue and ndma_state[0] % 2) else nc.sync
                        eng.dma_start(
                            out[it * 128:(it + 1) * 128,
                                d0 * 128:(d0 + nj) * 128],
                            osb[:, :])

            # sum-exp AllReduce split points: the bulk (A1) mid-pass-1, a
            # small A2, and a 2-tile B so the post-matmul tail is short.
            # Pass-2 groups are lists of dj indices per output chunk.
            if DT >= 16:
                ar_parts = [(0, 12), (12, 14), (14, 16)]
                groups = [list(range(4 * g, 4 * g + 4)) for g in range(3)] + \
                         [[12, 13], [14, 15]]
                interleave_at = {13: [groups[0]]}
                mid_groups = [groups[1], groups[2], groups[3]]
                b_groups = [groups[4]]
            else:
                ar_parts = [(0, DH)] + ([(DH, DT)] if DH < DT else [])
                groups = [list(range(g * GJ, (g + 1) * GJ)) for g in range(NG)]
                interleave_at = {}
                mid_groups = [g for g in groups if g[-1] < DH]
                b_groups = [g for g in groups if g[-1] >= DH]

            s_bounce = []
            for pi, (lo, hi) in enumerate(ar_parts):
                sin = dram_pool.tile([128, hi - lo], F32, name=f"s_in{pi}")
                sout = dram_pool.tile([128, hi - lo], F32, name=f"s_out{pi}")
                s_bounce.append((sin, sout))

            def ar_trigger(pi):
                lo, hi = ar_parts[pi]
                sin, sout = s_bounce[pi]
                nc.gpsimd.dma_start(sin[:, :], Sloc[:, lo:hi])
                nc.gpsimd.collective_compute(
                    "AllReduce", mybir.AluOpType.add,
                    replica_groups=[list(range(N_CORES_))],
                    ins=[sin.opt()], outs=[sout.opt()],
                )

            def ar_readback(pi):
                lo, hi = ar_parts[pi]
                sin, sout = s_bounce[pi]
                nc.gpsimd.dma_start(rv2[:, lo:hi], sout[:, :])
                nc.vector.tensor_scalar_mul(rv2[:, lo:hi], rv2[:, lo:hi],
                                            0.00006103515625)
                nc.vector.reciprocal(rv2[:, lo:hi], rv2[:, lo:hi])

            def emit_act(dj, srcs):
                for tc_i in range(TC):
                    st = st_pool.tile([128, 512], F32, name="st", tag="st")
                    nc.scalar.activation(
                        st[:, :], srcs[tc_i][:, :],
                        mybir.ActivationFunctionType.Tanh,
                        bias=qbv[:, dj:dj + 1], scale=1.0,
                    )
                    nc.scalar.activation(
                        e_tiles[dj][:, tc_i * 512:(tc_i + 1) * 512], st[:, :],
                        mybir.ActivationFunctionType.Exp,
                        bias=0.0, scale=vv[:, dj:dj + 1],
                        accum_out=acc[:, dj * TC + tc_i:dj * TC + tc_i + 1],
                    )
                nc.vector.tensor_reduce(
                    Sloc[:, dj:dj + 1],
                    acc[:, dj * TC:(dj + 1) * TC],
                    axis=mybir.AxisListType.X,
                    op=mybir.AluOpType.add,
                )
                inloop_parts = ar_parts[:-1] if len(ar_parts) > 1 else ar_parts
                for pi, (lo, hi) in enumerate(inloop_parts):
                    if dj == hi - 1:
                        ar_trigger(pi)
                        if pi == 0 and len(ar_parts) > 2:
                            pass  # readback deferred to the pass-2 start point
                        else:
                            ar_readback(pi)
                if len(ar_parts) > 2 and dj == ar_parts[1][1] - 1:
                    # just before the interleaved group: read back part 0
                    ar_readback(0)

            # dj 0..NSTG-1 evacuate PSUM to SBUF staging (no qbv dependency);
            # the q-projection matvec runs after dj NSTG-1's matmuls, by which
            # time its W1T tiles (loaded after VT) have arrived.
            NSTG = 2 if DT >= 8 else 0
            staged = []
            if NSTG == 0:
                with tc.tile_pool(name="w1pool", bufs=4) as w1_pool:
                    emit_matvec(w1_pool)
            for dj in range(DT):
                if dj < len(w2tb_pre):
                    w2tb = w2tb_pre[dj]
                else:
                    w2tb = w2tb_pool.tile([128, KT * 128], FP16, tag="w2tb",
                                          name="w2tb")
                    nc.sync.dma_start(w2tb[:, :], w2t[dj, :, :])
                ps_tiles = [psum_pool.tile([128, 512], F32, tag="ps", name=f"ps{i}")
                            for i in range(TC)]
                # k OUTER: stationary operand reused TC times; dj==0 streams
                # at VT-DMA pace.
                for kt in range(KT):
                    for tc_i in range(TC):
                        nc.tensor.matmul(
                            ps_tiles[tc_i][:, :],
                            w2tb[:, kt * 128:(kt + 1) * 128],
                            vt_tiles[kt][:, tc_i * 512:(tc_i + 1) * 512],
                            start=(kt == 0),
                            stop=(kt == KT - 1),
                        )
                if dj < NSTG:
                    sg = []
                    for tc_i in range(TC):
                        s = stg_pool.tile([128, 512], F32, tag="stg",
                                          name=f"sg{dj}_{tc_i}")
                        nc.scalar.copy(s[:, :], ps_tiles[tc_i][:, :])
                        sg.append(s)
                    staged.append((dj, sg))
                else:
                    emit_act(dj, ps_tiles)
                if dj == NSTG - 1 and NSTG > 0:
                    with tc.tile_pool(name="w1pool", bufs=4) as w1_pool:
                        emit_matvec(w1_pool)
                    for sdj, sg in staged:
                        emit_act(sdj, sg)
                for g in interleave_at.get(dj, []):
                    emit_group(g)

            # ---------------- last-part sum-exp AllReduce ---------------
            # Trigger immediately after the last dj's local reduce; run the
            # remaining earlier groups during its latency; read back and
            # reciprocal only after their DVE work is queued.
            if len(ar_parts) > 1:
                ar_trigger(len(ar_parts) - 1)

            if len(ar_parts) > 2:
                # mid_groups = [G1, G2, G3a]: G1/G2 need part 0 (ready),
                # G3a needs part 1
                for g in mid_groups[:-1]:
                    emit_group(g)
                ar_readback(1)
                emit_group(mid_groups[-1])
            else:
                for g in mid_groups:
                    emit_group(g)

            if len(ar_parts) > 1:
                ar_readback(len(ar_parts) - 1)

            for g in b_groups:
                emit_group(g, dual_issue=True)

    nc.compile()
    return nc


_NC_CACHE = None


def _get_nc():
    global _NC_CACHE
    if _NC_CACHE is None:
        _NC_CACHE = build_kernel()
    return _NC_CACHE


def make_in_maps(query, values, v, W1_w, W1_b, W2_w, W2_b,
                 D_=None, TS_=None, KS_=None, n_cores=N_CORES):
    import ml_dtypes
    D_ = D_ or D
    TS_ = TS_ or TS
    KS_ = KS_ or KS
    DT_ = D_ // 128
    KT_ = D_ // 128
    # W1T blocked: [kt, p, d] = W1_w[d, 128kt+p], fp8 (softmax is
    # invariant to the resulting per-column q_proj perturbation)
    w1t_blocked = np.ascontiguousarray(
        W1_w.T.reshape(KT_, 128, D_).astype(ml_dtypes.float8_e4m3))
    # w2t blocked: B[dj, p, kt, f] = W2_w[128dj+f, 128kt+p]
    w2t_blocked = np.ascontiguousarray(
        W2_w.reshape(DT_, 128, KT_, 128).transpose(0, 3, 2, 1)
        .reshape(DT_, 128, KT_ * 128).astype(np.float16))
    in_maps = []
    for c in range(n_cores):
        vs = np.ascontiguousarray(values[c * TS_:(c + 1) * TS_])
        vsT = np.ascontiguousarray(vs.T.astype(np.float16))
        in_maps.append({
            "valsT": vsT,
            "w2t": w2t_blocked,
            "w1t_d": w1t_blocked,
            "qfull": query,
            "w1b": W1_b,
            "w2b": W2_b,
            "vvec": v,
        })
    return in_maps


def kernel(query, values, v, W1_w, W1_b, W2_w, W2_b, _trace=False, _trace_kwargs=None):
    query = np.asarray(query, np.float32)
    values = np.asarray(values, np.float32)
    v = np.asarray(v, np.float32)
    W1_w = np.asarray(W1_w, np.float32)
    W1_b = np.asarray(W1_b, np.float32)
    W2_w = np.asarray(W2_w, np.float32)
    W2_b = np.asarray(W2_b, np.float32)

    nc = _get_nc()
    in_maps = make_in_maps(query, values, v, W1_w, W1_b, W2_w, W2_b)
    res = run_bass_kernel_spmd(
        nc, in_maps, core_ids=list(range(N_CORES)),
        trace=_trace, **(_trace_kwargs or {}),
    )
    shards = [np.asarray(om["out"], np.float32) for om in res.results]
    out = np.concatenate(shards, axis=0)
    if _trace:
        return out, res
    return out



# revision 15
# speedup vs baseline: 1.3434x; 1.3434x over previous
"""Bahdanau additive-attention kernel for Trainium2, SPMD across 8 NeuronCores.

Reference computation (all fp32):
    q_proj  = query @ W1_w.T + W1_b            # [D]
    v_proj  = values @ W2_w.T + W2_b           # [T, D]
    weights = softmax(tanh(q_proj + v_proj) * v, axis=0)   # over T
    out     = weights * values                 # [T, D]

Sharding: values is split along T across 8 cores (2048 rows each); W2/W1 are
replicated (shipped pre-transposed + pre-blocked in fp8e4m3, scaled by 64 to
stay out of fp8 subnormals); the softmax denominator (per-column sum of exps)
is AllReduced.  Logits are bounded in [-0.1, 0.1] (tanh * v with |v| <= 0.1)
so the softmax needs no max-subtraction pass.

Per-core device program:
  - Main matmul v_proj^T = W2T @ valuesT runs in fp8 DoubleRow perf mode
    (256-deep contraction per pass, 2x PE throughput): stationary w2 blocks
    [128, 2, 128], moving vt8 tiles [128, 2, 512]; psum = 64*v_proj in
    [d=128 part, t=512 free].
  - ScalarE: tanh(psum/64 + qb[d]) then exp(v[d] * x) with accum_out giving
    the per-partition running sum of exps (softmax denominator) for free.
  - e stored fp16 in SBUF (pre-scaled by 2^14/S at pass 2 so weights stay
    normal in fp16).  Pass 2: e *= 2^14/S[d], outT = e * valuesT(fp16) on
    DVE, TensorE transposes outT back to [t, d], VectorE descales+evacuates
    PSUM to SBUF f32, DMA out.
  - The q-projection matvec also runs fp8 DoubleRow, redundantly per core
    (cheap), scheduled after dj=1's matmuls so its W1 DMA hides behind the
    vt8 load.
"""

import numpy as np

import concourse.bacc as bacc
import concourse.bass as bass
import concourse.tile as tile
from concourse import mybir
from concourse import masks
from concourse.bass_utils import run_bass_kernel_spmd

F32 = mybir.dt.float32
BF16 = mybir.dt.bfloat16
FP16 = mybir.dt.float16
FP8 = mybir.dt.float8e4

D = 2048          # feature dim
T = 16384         # total timesteps
N_CORES = 8
TS = T // N_CORES  # timesteps per core = 2048

W_SCALE = 64.0           # host-side fp8 scale on W1/W2
INV_W_SCALE = 1.0 / W_SCALE


def build_kernel(D=D, TS=TS, n_cores=N_CORES, debug=False):
    DT = D // 128     # d-tiles of 128
    KT = D // 128     # k-tiles of 128
    KT2 = KT // 2     # k-tile PAIRS (DoubleRow consumes 256 contraction rows)
    TC = TS // 512    # t-chunks of 512
    IT = TS // 128    # t-tiles of 128
    GJ = min(4, DT)   # dj per pass-2 group (one 512-wide d-chunk)
    NG = DT // GJ     # number of pass-2 groups
    THW = min(1024, TS)  # pass-2 t-half width
    NTH = TS // THW
    N_CORES_ = n_cores
    DR = mybir.MatmulPerfMode.DoubleRow

    nc = bacc.Bacc(None, target_bir_lowering=False, debug=debug, num_devices=N_CORES_)

    # Per-core inputs (see make_in_maps for host-side layouts)
    valsT = nc.dram_tensor("valsT", [D, TS], FP16, kind="ExternalInput")
    valsT8 = nc.dram_tensor("valsT8", [KT2, 128, 2, TS], FP8, kind="ExternalInput")
    w2t8 = nc.dram_tensor("w2t8", [DT, 128, KT, 128], FP8, kind="ExternalInput")
    w1t8 = nc.dram_tensor("w1t8", [KT2, 128, 2, D], FP8, kind="ExternalInput")
    qfull = nc.dram_tensor("qfull", [D], F32, kind="ExternalInput")
    w1b = nc.dram_tensor("w1b", [D], F32, kind="ExternalInput")
    w2b = nc.dram_tensor("w2b", [D], F32, kind="ExternalInput")
    vvec = nc.dram_tensor("vvec", [D], F32, kind="ExternalInput")
    out = nc.dram_tensor("out", [TS, D], F32, kind="ExternalOutput")

    with tile.TileContext(nc) as tc:
        with (
            tc.tile_pool(name="const", bufs=1) as const_pool,
            tc.tile_pool(name="vt", bufs=1) as vt_pool,
            tc.tile_pool(name="vt8", bufs=1) as vt8_pool,
            tc.tile_pool(name="e", bufs=1) as e_pool,
            tc.tile_pool(name="w2tb", bufs=2) as w2tb_pool,
            tc.tile_pool(name="st", bufs=2) as st_pool,
            tc.tile_pool(name="outT", bufs=4) as outT_pool,
            tc.tile_pool(name="osb", bufs=5) as osb_pool,
            tc.tile_pool(name="stg", bufs=8) as stg_pool,
            tc.tile_pool(name="psum", bufs=6, space="PSUM") as psum_pool,
            tc.tile_pool(name="psumT", bufs=2, space="PSUM") as psumT_pool,
            tc.tile_pool(name="dram", bufs=1, space="DRAM") as dram_pool,
        ):
            # ---------------- constants / small vectors ----------------
            qbv = const_pool.tile([128, DT], F32)    # qb[d] laid out [p, dj]
            vv = const_pool.tile([128, DT], F32)     # v[d]
            rv2 = const_pool.tile([128, DT], F32)    # 2^14 / S[d]
            Sloc = const_pool.tile([128, DT], F32)   # local sum-exp
            b1v = const_pool.tile([128, DT], F32)
            b2v = const_pool.tile([128, DT], F32)
            acc = const_pool.tile([128, DT * TC], F32)  # per (dj, tc) exp-sums
            ident16 = const_pool.tile([128, 128], FP16)
            ones1 = const_pool.tile([1, 128], F32)
            qpart = const_pool.tile([128, DT], F32)  # local q_proj partial

            masks.make_identity(nc, ident16[:, :])
            nc.vector.memset(ones1[:, :], 1.0)

            DH = (3 * DT // 4) if DT >= 4 else DT

            # ---------------- warmup collective (absorbs ncfw first-use) ----
            wu_in = dram_pool.tile([1, 32], F32, name="wu_in")
            wu_out = dram_pool.tile([1, 32], F32, name="wu_out")
            wuz = const_pool.tile([1, 32], F32)
            nc.vector.memset(wuz[:, :], 0.0)
            nc.gpsimd.dma_start(wu_in[:, :], wuz[:, :])
            nc.gpsimd.collective_compute(
                "AllReduce", mybir.AluOpType.add,
                replica_groups=[list(range(N_CORES_))],
                ins=[wu_in.opt()], outs=[wu_out.opt()],
            )

            # first two W2T blocks land before the vt8 bulk so dj0 matmuls
            # can start immediately
            w2tb_pre = []
            for i in range(min(2, DT)):
                wpre = w2tb_pool.tile([128, KT, 128], FP8, tag="w2tb",
                                      name=f"w2tbp{i}")
                nc.sync.dma_start(wpre[:, :, :], w2t8[i, :, :, :])
                w2tb_pre.append(wpre)

            # ---------------- vt8 resident load (fp8, matmul moving op) ----
            # vt8[kt2][p, ks, t] = values_s[t, 256*kt2 + 128*ks + p]
            # Loaded in (kt2, tc) chunk order so dj0's matmuls stream at DMA
            # pace.
            vt8_tiles = []
            for kt2 in range(KT2):
                vt8t = vt8_pool.tile([128, 2, TS], FP8, name=f"vt8_{kt2}")
                vt8_tiles.append(vt8t)
            for kt2 in range(KT2):
                for tcq in range(2):
                    hw = TS // 2
                    nc.sync.dma_start(
                        vt8_tiles[kt2][:, :, tcq * hw:(tcq + 1) * hw],
                        valsT8[kt2, :, :, tcq * hw:(tcq + 1) * hw])

            # ---------------- pass 1: matmul + tanh + exp ---------------
            e_tiles = []
            for dj in range(DT):
                e_tiles.append(e_pool.tile([128, TS], FP16, name=f"e{dj}"))

            # vt16 (pass-2 fp16 values) trickle-loaded one tile per dj
            # iteration on gpsimd — needed only from the first pass-2 group.
            vt_tiles = []
            for kt in range(KT):
                vt = vt_pool.tile([128, TS], FP16, name=f"vt{kt}")
                vt_tiles.append(vt)

            qcol = const_pool.tile([128, KT], F32)   # q in [p, kt] layout
            # q in fp8, one value per 16B block: dual-fp8 LDWEIGHTS requires
            # the pair stride to be even and 16B-aligned, so the (ks=0, ks=1)
            # elements of a DoubleRow pair sit 16 bytes apart.
            qcol16 = const_pool.tile([128, KT * 16], FP8)
            qrow = const_pool.tile([1, D], F32)      # q_proj as a row
            nc.gpsimd.dma_start(qcol[:, :], qfull[:].rearrange("(kt p) -> p kt", p=128))
            nc.vector.tensor_copy(
                qcol16[:, :].rearrange("p (a b) -> p a b", b=16)[:, :, 0:1],
                qcol[:, :].rearrange("p (a b) -> p a b", b=1))
            QW = min(512, D)
            QDC = D // QW

            def emit_matvec(w1_pool):
                # q_proj row = sum_kt2 q_pair.T @ W1T[pair] in fp8 DoubleRow
                # (softmax is nearly invariant to the per-column q_proj
                # quantization error), then transpose the row into the
                # per-partition [p, dj] layout.  W1 DMAs ride sync AFTER the
                # vt8 bulk, so they never steal HBM bandwidth from pass 1.
                pq_tiles = [psum_pool.tile([1, QW], F32, name=f"pq{dc}", tag="ps")
                            for dc in range(QDC)]
                for kt2 in range(KT2):
                    w1tile = w1_pool.tile([128, 2, D], FP8, tag="w1t")
                    nc.sync.dma_start(w1tile[:, :, :], w1t8[kt2, :, :, :])
                    qpair = qcol16[:, :].rearrange(
                        "p (a b) -> p a b", b=16)[:, 2 * kt2:2 * kt2 + 2, 0:1]
                    for dc in range(QDC):
                        nc.tensor.matmul(
                            pq_tiles[dc][:, :],
                            qpair,
                            w1tile[:, :, dc * QW:(dc + 1) * QW],
                            start=(kt2 == 0), stop=(kt2 == KT2 - 1),
                            perf_mode=DR)
                for dc in range(QDC):
                    nc.scalar.activation(
                        qrow[:, dc * QW:(dc + 1) * QW], pq_tiles[dc][:, :],
                        mybir.ActivationFunctionType.Copy,
                        bias=0.0, scale=INV_W_SCALE)
                pqt = psumT_pool.tile([128, DT], F32, name="pqt", tag="pT")
                for dj in range(DT):
                    nc.tensor.transpose(
                        pqt[:, dj:dj + 1],
                        qrow[:, dj * 128:(dj + 1) * 128], ones1[:, 0:1])
                nc.scalar.copy(qbv[:, :], pqt[:, :])

                # biases / v in [p, dj] layout: elem (p, j) <- dram[128j + p]
                nc.gpsimd.dma_start(b1v[:, :], w1b[:].rearrange("(j p) -> p j", p=128))
                nc.gpsimd.dma_start(b2v[:, :], w2b[:].rearrange("(j p) -> p j", p=128))
                nc.gpsimd.dma_start(vv[:, :], vvec[:].rearrange("(j p) -> p j", p=128))
                nc.vector.tensor_add(b1v[:, :], b1v[:, :], b2v[:, :])
                nc.vector.tensor_add(qbv[:, :], qbv[:, :], b1v[:, :])

            ndma_state = [0]

            def emit_group(djs, dual_issue=False):
                # pass-2 pipeline for a list of dj tiles (one contiguous
                # output chunk): scale e by 2^14/S, multiply with resident
                # fp16 VT, transpose on TensorE, descale-evacuate on VectorE.
                nj = len(djs)
                d0 = djs[0]
                for th in range(NTH):
                    oT = []
                    for jj in range(nj):
                        dj = djs[jj]
                        if th == 0:
                            nc.vector.tensor_scalar(
                                out=e_tiles[dj][:, :], in0=e_tiles[dj][:, :],
                                scalar1=rv2[:, dj:dj + 1], scalar2=None,
                                op0=mybir.AluOpType.mult)
                        ot = outT_pool.tile([128, THW], FP16, tag="oT", name="ot")
                        nc.vector.tensor_mul(
                            ot[:, :],
                            e_tiles[dj][:, th * THW:(th + 1) * THW],
                            vt_tiles[dj][:, th * THW:(th + 1) * THW])
                        oT.append(ot)
                    for itl in range(THW // 128):
                        it = th * (THW // 128) + itl
                        pst = psumT_pool.tile([128, nj * 128], FP16, tag="pT",
                                              name="pst")
                        for jj in range(nj):
                            nc.tensor.transpose(
                                pst[:, jj * 128:(jj + 1) * 128],
                                oT[jj][:, itl * 128:(itl + 1) * 128],
                                ident16[:, :],
                            )
                        osb = osb_pool.tile([128, nj * 128], F32, name="osb",
                                            tag="osb")
                        nc.vector.tensor_scalar_mul(
                            osb[:, :], pst[:, :], 0.00006103515625)
                        ndma_state[0] += 1
                        eng = nc.gpsimd if (dual_issue and ndma_state[0] % 2) else nc.sync
                        eng.dma_start(
                            out[it * 128:(it + 1) * 128,
                                d0 * 128:(d0 + nj) * 128],
                            osb[:, :])

            # sum-exp AllReduce split points: the bulk (A1) mid-pass-1, a
            # small A2, and a 2-tile B so the post-matmul tail is short.
            # Pass-2 groups are lists of dj indices per output chunk.
            if DT >= 16:
                ar_parts = [(0, 12), (12, 14), (14, 16)]
                groups = [list(range(4 * g, 4 * g + 4)) for g in range(3)] + \
                         [[12, 13], [14, 15]]
                interleave_at = {13: [groups[0]]}
                mid_groups = [groups[1], groups[2], groups[3]]
                b_groups = [groups[4]]
            else:
                ar_parts = [(0, DH)] + ([(DH, DT)] if DH < DT else [])
                groups = [list(range(g * GJ, (g + 1) * GJ)) for g in range(NG)]
                interleave_at = {}
                mid_groups = [g for g in groups if g[-1] < DH]
                b_groups = [g for g in groups if g[-1] >= DH]

            s_bounce = []
            for pi, (lo, hi) in enumerate(ar_parts):
                sin = dram_pool.tile([128, hi - lo], F32, name=f"s_in{pi}")
                sout = dram_pool.tile([128, hi - lo], F32, name=f"s_out{pi}")
                s_bounce.append((sin, sout))

            def ar_trigger(pi):
                lo, hi = ar_parts[pi]
                sin, sout = s_bounce[pi]
                nc.gpsimd.dma_start(sin[:, :], Sloc[:, lo:hi])
                nc.gpsimd.collective_compute(
                    "AllReduce", mybir.AluOpType.add,
                    replica_groups=[list(range(N_CORES_))],
                    ins=[sin.opt()], outs=[sout.opt()],
                )

            def ar_readback(pi):
                lo, hi = ar_parts[pi]
                sin, sout = s_bounce[pi]
                nc.gpsimd.dma_start(rv2[:, lo:hi], sout[:, :])
                nc.vector.tensor_scalar_mul(rv2[:, lo:hi], rv2[:, lo:hi],
                                            0.00006103515625)
                nc.vector.reciprocal(rv2[:, lo:hi], rv2[:, lo:hi])

            def emit_act(dj, srcs):
                for tc_i in range(TC):
                    st = st_pool.tile([128, 512], FP16, name="st", tag="st")
                    nc.scalar.activation(
                        st[:, :], srcs[tc_i][:, :],
                        mybir.ActivationFunctionType.Tanh,
                        bias=qbv[:, dj:dj + 1], scale=INV_W_SCALE,
                    )
                    nc.scalar.activation(
                        e_tiles[dj][:, tc_i * 512:(tc_i + 1) * 512], st[:, :],
                        mybir.ActivationFunctionType.Exp,
                        bias=0.0, scale=vv[:, dj:dj + 1],
                        accum_out=acc[:, dj * TC + tc_i:dj * TC + tc_i + 1],
                    )
                nc.vector.tensor_reduce(
                    Sloc[:, dj:dj + 1],
                    acc[:, dj * TC:(dj + 1) * TC],
                    axis=mybir.AxisListType.X,
                    op=mybir.AluOpType.add,
                )
                inloop_parts = ar_parts[:-1] if len(ar_parts) > 1 else ar_parts
                for pi, (lo, hi) in enumerate(inloop_parts):
                    if dj == hi - 1:
                        ar_trigger(pi)
                        if pi == 0 and len(ar_parts) > 2:
                            pass  # readback deferred to the pass-2 start point
                        else:
                            ar_readback(pi)
                if len(ar_parts) > 2 and dj == ar_parts[1][1] - 1:
                    # just before the interleaved group: read back part 0
                    ar_readback(0)

            # dj 0..NSTG-1 evacuate PSUM to SBUF staging (no qbv dependency);
            # the q-projection matvec runs after dj NSTG-1's matmuls, by which
            # time its W1 tiles (loaded after vt8) have arrived.
            NSTG = 2 if DT >= 8 else 0
            staged = []
            if NSTG == 0:
                with tc.tile_pool(name="w1pool", bufs=2) as w1_pool:
                    emit_matvec(w1_pool)
            for dj in range(DT):
                if dj < len(w2tb_pre):
                    w2tb = w2tb_pre[dj]
                else:
                    w2tb = w2tb_pool.tile([128, KT, 128], FP8, tag="w2tb",
                                          name="w2tb")
                    nc.gpsimd.dma_start(w2tb[:, :, :], w2t8[dj, :, :, :])
                # vt16 trickle (pass-2 fp16 values): two tiles per dj from
                # dj=2 on, so it never front-runs the vt8/W1 loads.
                if 2 <= dj < 2 + KT // 2:
                    for h in range(2):
                        kt = 2 * (dj - 2) + h
                        nc.gpsimd.dma_start(
                            vt_tiles[kt][:, :],
                            valsT[kt * 128:(kt + 1) * 128, :])
                ps_tiles = [psum_pool.tile([128, 512], F32, tag="ps", name=f"ps{i}")
                            for i in range(TC)]
                # kt2 OUTER: stationary pair reused TC times; dj==0 streams
                # at vt8-DMA pace.  DoubleRow: 256-deep contraction per pass.
                for kt2 in range(KT2):
                    for tc_i in range(TC):
                        nc.tensor.matmul(
                            ps_tiles[tc_i][:, :],
                            w2tb[:, 2 * kt2:2 * kt2 + 2, :],
                            vt8_tiles[kt2][:, :, tc_i * 512:(tc_i + 1) * 512],
                            start=(kt2 == 0),
                            stop=(kt2 == KT2 - 1),
                            perf_mode=DR,
                        )
                if dj < NSTG:
                    sg = []
                    for tc_i in range(TC):
                        s = stg_pool.tile([128, 512], FP8, tag="stg",
                                          name=f"sg{dj}_{tc_i}")
                        nc.scalar.copy(s[:, :], ps_tiles[tc_i][:, :])
                        sg.append(s)
                    staged.append((dj, sg))
                else:
                    emit_act(dj, ps_tiles)
                if dj == NSTG - 1 and NSTG > 0:
                    with tc.tile_pool(name="w1pool", bufs=2) as w1_pool:
                        emit_matvec(w1_pool)
                    for sdj, sg in staged:
                        emit_act(sdj, sg)
                for g in interleave_at.get(dj, []):
                    emit_group(g)

            # ---------------- last-part sum-exp AllReduce ---------------
            # Trigger immediately after the last dj's local reduce; run the
            # remaining earlier groups during its latency; read back and
            # reciprocal only after their DVE work is queued.
            if len(ar_parts) > 1:
                ar_trigger(len(ar_parts) - 1)

            if len(ar_parts) > 2:
                # mid_groups = [G1, G2, G3a]: G1/G2 need part 0 (ready),
                # G3a needs part 1
                for g in mid_groups[:-1]:
                    emit_group(g)
                ar_readback(1)
                emit_group(mid_groups[-1])
            else:
                for g in mid_groups:
                    emit_group(g)

            if len(ar_parts) > 1:
                ar_readback(len(ar_parts) - 1)

            for g in b_groups:
                emit_group(g, dual_issue=True)

    nc.compile()
    return nc


_NC_CACHE = None


def _get_nc():
    global _NC_CACHE
    if _NC_CACHE is None:
        _NC_CACHE = build_kernel()
    return _NC_CACHE


def make_in_maps(query, values, v, W1_w, W1_b, W2_w, W2_b,
                 D_=None, TS_=None, n_cores=N_CORES):
    import ml_dtypes
    D_ = D_ or D
    TS_ = TS_ or TS
    DT_ = D_ // 128
    KT_ = D_ // 128
    KT2_ = KT_ // 2
    fp8 = ml_dtypes.float8_e4m3
    # W1T DoubleRow pairs: [kt2, p, ks, d] = 64*W1_w[d, 256kt2+128ks+p]
    w1t_blocked = np.ascontiguousarray(
        (W1_w.T * W_SCALE).reshape(KT2_, 2, 128, D_).transpose(0, 2, 1, 3)
        .astype(fp8))
    # w2t blocked: B[dj, p, kt, f] = 64*W2_w[128dj+f, 128kt+p]
    # (pairs of k-blocks are adjacent along the kt dim => DoubleRow-ready)
    w2t_blocked = np.ascontiguousarray(
        (W2_w * W_SCALE).reshape(DT_, 128, KT_, 128).transpose(0, 3, 2, 1)
        .astype(fp8))
    in_maps = []
    for c in range(n_cores):
        vs = np.ascontiguousarray(values[c * TS_:(c + 1) * TS_])
        vsT = np.ascontiguousarray(vs.T.astype(np.float16))
        vsT8 = np.ascontiguousarray(
            vs.T.astype(fp8).reshape(KT2_, 2, 128, TS_).transpose(0, 2, 1, 3))
        in_maps.append({
            "valsT": vsT,
            "valsT8": vsT8,
            "w2t8": w2t_blocked,
            "w1t8": w1t_blocked,
            "qfull": query,
            "w1b": W1_b,
            "w2b": W2_b,
            "vvec": v,
        })
    return in_maps


def kernel(query, values, v, W1_w, W1_b, W2_w, W2_b, _trace=False, _trace_kwargs=None):
    query = np.asarray(query, np.float32)
    values = np.asarray(values, np.float32)
    v = np.asarray(v, np.float32)
    W1_w = np.asarray(W1_w, np.float32)
    W1_b = np.asarray(W1_b, np.float32)
    W2_w = np.asarray(W2_w, np.float32)
    W2_b = np.asarray(W2_b, np.float32)

    nc = _get_nc()
    in_maps = make_in_maps(query, values, v, W1_w, W1_b, W2_w, W2_b)
    res = run_bass_kernel_spmd(
        nc, in_maps, core_ids=list(range(N_CORES)),
        trace=_trace, **(_trace_kwargs or {}),
    )
    shards = [np.asarray(om["out"], np.float32) for om in res.results]
    out = np.concatenate(shards, axis=0)
    if _trace:
        return out, res
    return out
